# revision 5
# baseline (speedup 1.0000x reference)
"""Trainium2 Bass kernel for batched multi-head attention — v4.

Problem: q,k,v [B=2, H=16, S=2048, D=64] fp32 ->
         out[b,h,i,d] = softmax(q @ k^T / sqrt(D), axis=-1) @ v

Sharding: 32 (b,h) pairs split across 8 NeuronCores, 4 heads/core, SPMD,
no cross-core communication.

v4 = v3 with the I/O tail consolidated (measured per-instruction costs
of this backend are ~flat, so fewer instructions win):
  - One input DMA per head: host packs [QT | KT | V''] into a single
    fp16 [128, 3S] SBUF image (QT/KT: d-on-partitions, zero-padded to
    128; V'': [V | ones | zeros] blocks with j%128 on partitions).
  - One [128, 2048] PSUM accumulator per head (AV matmuls write
    512-aligned column slices, so start=True clears exactly one bank),
    evacuated by a single full-tile scalar copy and a single DMA.
  - Scores: ST[j,i] = KT_blk^T @ QT, 4x [K=128,M=128,N=512] fp16 MMs
    per (h, jblk) -> PSUM fp32; exp on ScalarE -> bf16 ET (fp16-dest
    activations are ~4-10x slower on this backend); AV V''-stationary
    fp16-lhsT x bf16-rhs MMs accumulate over jblk.
  - Host epilogue: divide by the denominator row (from the ones column)
    and transpose [d, i] -> [i, d] in numpy.
"""

import numpy as np

B, H, S, D = 2, 16, 2048, 64
N_CORES = 8
HL = (B * H) // N_CORES          # 4 local heads per core
NJ = S // 128                    # 16 key blocks
CH = 512                         # matmul N / psum bank width
NCH = S // CH                    # 4 chunks
E = D + 1                        # useful output rows (64 dims + denom)

_CACHE = {}


def _build(repeat=1):
    import concourse.tile as tile
    from concourse import bacc, mybir

    fp32 = mybir.dt.float32
    fp16 = mybir.dt.float16
    bf16 = mybir.dt.bfloat16
    Exp = mybir.ActivationFunctionType.Exp

    nc = bacc.Bacc("TRN2", target_bir_lowering=False, debug=False)
    qkv_d = nc.dram_tensor("qkv", [HL, 128, 3 * S], fp16,
                           kind="ExternalInput").ap()
    o_d = nc.dram_tensor("ot", [HL, E, S], fp32, kind="ExternalOutput").ap()

    with tile.TileContext(nc) as tc:
        import contextlib
        ctx = contextlib.ExitStack()
        with ctx:
            p_in = ctx.enter_context(tc.tile_pool(name="p_in", bufs=HL))
            p_e = ctx.enter_context(tc.tile_pool(name="p_e", bufs=2))
            p_sp = ctx.enter_context(tc.tile_pool(name="p_sp", bufs=1, space="PSUM"))
            p_oa = ctx.enter_context(tc.tile_pool(name="p_oa", bufs=1, space="PSUM"))
            p_os = ctx.enter_context(tc.tile_pool(name="p_os", bufs=2))

            for rep in range(repeat):
                QKV = {}
                for h in range(HL):
                    QKV[h] = p_in.tile([128, 3 * S], fp16, tag="qkv",
                                       name=f"qkv{rep}_{h}")
                    nc.sync.dma_start(out=QKV[h][:], in_=qkv_d[h])

                for h in range(HL):
                    qt = QKV[h][:, 0:S]
                    kt = QKV[h][:, S:2 * S]
                    vt = QKV[h][:, 2 * S:3 * S]
                    oat = p_oa.tile([128, S], fp32, tag="oa", name=f"oa{rep}_{h}")
                    for j in range(NJ):
                        sp = p_sp.tile([128, S], fp32, tag="sp",
                                       name=f"sp{rep}_{h}_{j}")
                        for c in range(NCH):
                            nc.tensor.matmul(
                                sp[:, c * CH:(c + 1) * CH],
                                lhsT=kt[:, j * 128:(j + 1) * 128],
                                rhs=qt[:, c * CH:(c + 1) * CH],
                                start=True, stop=True,
                            )
                        et = p_e.tile([128, S], bf16, tag="et",
                                      name=f"et{rep}_{h}_{j}")
                        nc.scalar.activation(et[:], sp[:], Exp,
                                             scale=float(D) ** -0.5)
                        for c in range(NCH):
                            nc.tensor.matmul(
                                oat[:, c * CH:(c + 1) * CH],
                                lhsT=vt[:, j * 128:(j + 1) * 128],
                                rhs=et[:, c * CH:(c + 1) * CH],
                                start=(j == 0), stop=(j == NJ - 1),
                            )
                    os_t = p_os.tile([128, S], fp32, tag="os", name=f"os{rep}_{h}")
                    nc.scalar.copy(os_t[:], oat[:])
                    nc.sync.dma_start(out=o_d[h], in_=os_t[0:E, :])

    nc.compile()
    return nc


def _get_nc():
    if "nc" not in _CACHE:
        _CACHE["nc"] = _build()
    return _CACHE["nc"]


def _spec_kw(spec):
    return {}


def _prep_core(q, k, v):
    """q,k,v: [HL, S, D] fp32 -> qkv [HL, 128, 3S] fp16 SBUF image."""
    qkv = np.zeros((HL, 128, 3 * S), dtype=np.float16)
    qkv[:, :D, 0:S] = q.transpose(0, 2, 1).astype(np.float16)
    qkv[:, :D, S:2 * S] = k.transpose(0, 2, 1).astype(np.float16)
    vv = np.zeros((HL, S, 128), dtype=np.float16)
    vv[:, :, :D] = v.astype(np.float16)
    vv[:, :, D] = 1.0
    qkv[:, :, 2 * S:3 * S] = vv.reshape(HL, NJ, 128, 128).transpose(0, 2, 1, 3) \
                               .reshape(HL, 128, S)
    return qkv


def _in_maps(q, k, v, **_kw):
    maps = []
    for c in range(N_CORES):
        sl = slice(c * HL, (c + 1) * HL)
        maps.append({"qkv": _prep_core(q[sl], k[sl], v[sl])})
    return maps


def kernel(q, k, v):
    from concourse.bass_utils import run_bass_kernel_spmd

    q = np.asarray(q, dtype=np.float32).reshape(B * H, S, D)
    k = np.asarray(k, dtype=np.float32).reshape(B * H, S, D)
    v = np.asarray(v, dtype=np.float32).reshape(B * H, S, D)

    nc = _get_nc()
    res = run_bass_kernel_spmd(nc, _in_maps(q, k, v), list(range(N_CORES)))

    out = np.empty((B * H, S, D), dtype=np.float32)
    for c in range(N_CORES):
        ot = res.results[c]["ot"]            # [HL, 65, S] fp32
        o = ot[:, :D, :] / ot[:, D:D + 1, :]
        out[c * HL:(c + 1) * HL] = o.transpose(0, 2, 1)
    return out.reshape(B, H, S, D)


if __name__ == "__main__":
    rng = np.random.default_rng(0)
    q = rng.standard_normal((B, H, S, D), dtype=np.float32)
    k = rng.standard_normal((B, H, S, D), dtype=np.float32)
    v = rng.standard_normal((B, H, S, D), dtype=np.float32)
    out = kernel(q, k, v)
    errs = []
    for b in range(B):
        for h in range(H):
            s = (q[b, h] @ k[b, h].T) * D ** -0.5
            e = np.exp(s - s.max(-1, keepdims=True))
            want = (e / e.sum(-1, keepdims=True)) @ v[b, h]
            errs.append(np.abs(out[b, h] - want).max() / np.abs(want).max())
    print(f"max head rel err: {max(errs):.3e}")
